# revision 8
# baseline (speedup 1.0000x reference)
"""GPTSAN attention kernel for 8 Trainium2 NeuronCores.

Sharding: core c handles batch b = c // 4 and heads [4*(c%4), 4*(c%4)+4).
Each core computes QKV projection, causal attention and a partial output
projection for its 4 heads; the host sums the 4 partials per batch.

On-chip layout is fully transposed (QT/KT = [head*dk, seq]) so no transposes
are needed on the hot path:
  - QT/KT from matmul(lhsT=W chunk, rhs=XT chunk)
  - V in natural [seq, head*dk] from matmul(lhsT=XT chunk, rhs=Wv)
  - scoresT[m, r] per 128-row block, causally skipping blocks with m > r
  - exp(0.125*s) on ScalarE (scores are bounded, no max subtraction; masked
    entries of the reference underflow to exactly 0, so skipping them and
    zeroing the diagonal block's upper triangle is exact)
  - PV with a ones column appended to V yields row sums for free; the
    reciprocal feeds ScalarE's per-partition scale to normalize; a PE
    transpose returns attn to [head*dk, seq] for the output projection.
"""

import numpy as np
import ml_dtypes

B = 2
S = 2048
D = 1024
H = 16
DK = 64
NCORES = 8
HPC = H // NCORES * 2  # 4 heads per core (2 batches * 8 cores / 16 heads)
HD = HPC * DK  # 256
RB = S // 128  # 16 row blocks
KC = D // 128  # 8 contraction chunks for the projections

_compiled = None


def _build():
    import concourse.tile as tile
    from concourse import bacc, mybir

    fp32 = mybir.dt.float32
    bf16 = mybir.dt.bfloat16
    Exp = mybir.ActivationFunctionType.Exp
    Copy = mybir.ActivationFunctionType.Copy

    nc = bacc.Bacc()

    xt = nc.declare_dram_parameter("xt", [D, S], bf16, isOutput=False)
    wqkv = nc.declare_dram_parameter("wqkv", [D, 3 * HD], bf16, isOutput=False)
    wo = nc.declare_dram_parameter("wo", [HD, D], bf16, isOutput=False)
    triu = nc.declare_dram_parameter("triu", [128, 128], bf16, isOutput=False)
    ident = nc.declare_dram_parameter("ident", [128, 128], bf16, isOutput=False)
    kt_out = nc.declare_dram_parameter("kt_out", [HD, S], fp32, isOutput=True)
    v_out = nc.declare_dram_parameter("v_out", [128, RB * HD], fp32, isOutput=True)
    ft_out = nc.declare_dram_parameter("ft_out", [D, S], fp32, isOutput=True)

    with tile.TileContext(nc) as tc:
        with (
            tc.tile_pool(name="const", bufs=1) as cpool,
            tc.tile_pool(name="big", bufs=1) as bigpool,
            tc.tile_pool(name="work", bufs=2) as wpool,
            tc.tile_pool(name="stage", bufs=4) as spool,
            tc.tile_pool(name="mm", bufs=3, space="PSUM") as mmpool,
            tc.tile_pool(name="pv", bufs=2, space="PSUM") as pvpool,
            tc.tile_pool(name="tr", bufs=2, space="PSUM") as trpool,
        ):
            # ---- resident tiles ----
            xt_sb = bigpool.tile([128, KC, S], bf16)        # X^T, d-chunked
            w_sb = cpool.tile([128, KC, 3 * HD], bf16)      # qkv weights
            wo_sb = cpool.tile([128, 2, D], bf16)           # o weights, hd-chunked
            triu_sb = cpool.tile([128, 128], bf16)
            id_sb = cpool.tile([128, 128], bf16)
            qt_sb = bigpool.tile([128, 2, S], bf16)         # Q^T  (hd-chunked)
            ktb_sb = bigpool.tile([128, 2, S], bf16)        # K^T  bf16
            ktf_sb = bigpool.tile([128, 2, S], fp32)        # K^T  fp32 (output)
            vf_sb = bigpool.tile([128, RB * HD], fp32)      # V natural fp32 (output)
            vplus_sb = bigpool.tile([128, RB, HPC, DK + 1], bf16)  # V + ones col
            attnt_sb = bigpool.tile([128, 2, S], bf16)      # attn^T

            nc.sync.dma_start(out=xt_sb[:, :, :], in_=xt.rearrange("(c p) s -> p c s", p=128))
            nc.sync.dma_start(out=w_sb[:, :, :], in_=wqkv.rearrange("(c p) n -> p c n", p=128))
            nc.sync.dma_start(out=wo_sb[:, :, :], in_=wo.rearrange("(c p) n -> p c n", p=128))
            nc.sync.dma_start(out=triu_sb[:, :], in_=triu[:, :])
            nc.sync.dma_start(out=id_sb[:, :], in_=ident[:, :])
            nc.vector.memset(vplus_sb[:, :, :, :], 1.0)
            # pre-touch triu on DVE so the diag-mask ops (whose ISA struct
            # allows a single sync wait) never need a DMA wait
            scratch = cpool.tile([128, 128], bf16)
            nc.vector.tensor_copy(out=scratch[:, :], in_=triu_sb[:, :])

            # ---- phase A: projections ----
            # Q^T and K^T: stationary = weight chunk, moving = X^T chunk
            for which in range(2):  # 0 = Q, 1 = K
                for hdc in range(2):
                    for sc in range(4):
                        ps = mmpool.tile([128, 512], fp32, tag="mm")
                        for kc in range(KC):
                            nc.tensor.matmul(
                                out=ps[:, :],
                                lhsT=w_sb[:, kc, which * HD + hdc * 128:which * HD + hdc * 128 + 128],
                                rhs=xt_sb[:, kc, sc * 512:sc * 512 + 512],
                                start=(kc == 0),
                                stop=(kc == KC - 1),
                            )
                        if which == 0:
                            nc.vector.tensor_copy(out=qt_sb[:, hdc, sc * 512:sc * 512 + 512], in_=ps[:, :])
                        else:
                            nc.vector.tensor_copy(out=ktb_sb[:, hdc, sc * 512:sc * 512 + 512], in_=ps[:, :])
                            nc.scalar.activation(
                                out=ktf_sb[:, hdc, sc * 512:sc * 512 + 512], in_=ps[:, :], func=Copy)
            nc.sync.dma_start(out=kt_out.rearrange("(c p) s -> p c s", p=128), in_=ktf_sb[:, :, :])

            # V natural: stationary = X^T chunk, moving = Wv
            for mc in range(RB):
                ps = mmpool.tile([128, HD], fp32, tag="mm")
                for kc in range(KC):
                    nc.tensor.matmul(
                        out=ps[:, :],
                        lhsT=xt_sb[:, kc, mc * 128:mc * 128 + 128],
                        rhs=w_sb[:, kc, 2 * HD:3 * HD],
                        start=(kc == 0),
                        stop=(kc == KC - 1),
                    )
                nc.vector.tensor_copy(out=vf_sb[:, mc * HD:mc * HD + HD], in_=ps[:, :])
                for h in range(HPC):
                    nc.vector.tensor_copy(
                        out=vplus_sb[:, mc, h, 0:DK], in_=ps[:, h * DK:h * DK + DK])
            nc.sync.dma_start(out=v_out[:, :], in_=vf_sb[:, :])

            # ---- phase B: attention ----
            for h in range(HPC):
                hdc = h // 2
                po = 64 * (h % 2)
                for r in range(RB):
                    probs = wpool.tile([128, S], bf16, tag="probs")
                    nmc = r + 1
                    for g in range((nmc + 3) // 4):
                        mcs = list(range(4 * g, min(4 * g + 4, nmc)))
                        ps = mmpool.tile([128, 512], fp32, tag="mm")
                        for j, mc in enumerate(mcs):
                            nc.tensor.matmul(
                                out=ps[:, j * 128:j * 128 + 128],
                                lhsT=ktb_sb[po:po + 64, hdc, mc * 128:mc * 128 + 128],
                                rhs=qt_sb[po:po + 64, hdc, r * 128:r * 128 + 128],
                                start=True,
                                stop=True,
                            )
                        w = 128 * len(mcs)
                        nc.scalar.activation(
                            out=probs[:, 512 * g:512 * g + w], in_=ps[:, 0:w],
                            func=Exp, scale=0.125)
                    # causal mask on the diagonal block: zero entries with m > r
                    diag = wpool.tile([128, 128], bf16, tag=f"diag{h}_{r}")
                    # probs_diag * triu via the TensorScalarPtr struct (the
                    # plain TensorTensor struct only supports one sync wait)
                    nc.vector.scalar_tensor_tensor(
                        out=diag[:, :],
                        in0=probs[:, r * 128:r * 128 + 128],
                        scalar=1.0,
                        in1=triu_sb[:, :],
                        op0=mybir.AluOpType.mult,
                        op1=mybir.AluOpType.mult,
                    )
                    pvp = pvpool.tile([128, DK + 1], fp32, tag="pv")
                    for mc in range(nmc):
                        nc.tensor.matmul(
                            out=pvp[:, :],
                            lhsT=(diag[:, :] if mc == r
                                  else probs[:, mc * 128:mc * 128 + 128]),
                            rhs=vplus_sb[:, mc, h, :],
                            start=(mc == 0),
                            stop=(mc == nmc - 1),
                        )
                    recip = spool.tile([128, 1], fp32, tag="recip")
                    nc.vector.reciprocal(out=recip[:, :], in_=pvp[:, DK:DK + 1])
                    anat = spool.tile([128, DK], bf16, tag="anat")
                    nc.scalar.activation(
                        out=anat[:, :], in_=pvp[:, 0:DK], func=Copy, scale=recip[:, :])
                    trp = trpool.tile([64, 128], bf16, tag="tr")
                    nc.tensor.transpose(out=trp[:, :], in_=anat[:, :], identity=id_sb[:, :])
                    nc.vector.tensor_copy(
                        out=attnt_sb[po:po + 64, hdc, r * 128:r * 128 + 128], in_=trp[:, :])

            # ---- phase C: output projection (partial; host sums over cores) ----
            for cc in range(KC):
                for sc in range(4):
                    ps = mmpool.tile([128, 512], fp32, tag="mm")
                    for hdc in range(2):
                        nc.tensor.matmul(
                            out=ps[:, :],
                            lhsT=wo_sb[:, hdc, cc * 128:cc * 128 + 128],
                            rhs=attnt_sb[:, hdc, sc * 512:sc * 512 + 512],
                            start=(hdc == 0),
                            stop=(hdc == 1),
                        )
                    fst = spool.tile([128, 512], fp32, tag="fst")
                    nc.vector.tensor_copy(out=fst[:, :], in_=ps[:, :])
                    nc.sync.dma_start(
                        out=ft_out[cc * 128:cc * 128 + 128, sc * 512:sc * 512 + 512],
                        in_=fst[:, :])

    nc.finalize()
    return nc


def kernel(hidden_states, mask, qkv, o):
    global _compiled
    from concourse.bass_utils import run_bass_kernel_spmd

    if _compiled is None:
        _compiled = _build()
    nc = _compiled

    bf = ml_dtypes.bfloat16
    tri = mask[0, :128, :128].T.astype(bf)          # triu incl. diagonal
    ident = np.eye(128, dtype=np.float32).astype(bf)

    in_maps = []
    for c in range(NCORES):
        b = c // 4
        g = c % 4
        hs = slice(4 * g, 4 * g + 4)
        xt = np.ascontiguousarray(hidden_states[b].T).astype(bf)          # [D, S]
        wq = qkv[:, 0, hs, :].reshape(D, HD)
        wk = qkv[:, 1, hs, :].reshape(D, HD)
        wv = qkv[:, 2, hs, :].reshape(D, HD)
        wqkv_c = np.concatenate([wq, wk, wv], axis=1).astype(bf)          # [D, 768]
        wo_c = np.ascontiguousarray(o[hs].reshape(HD, D)).astype(bf)      # [256, D]
        in_maps.append({
            "xt": xt, "wqkv": np.ascontiguousarray(wqkv_c),
            "wo": wo_c, "triu": np.ascontiguousarray(tri), "ident": ident,
        })

    res = run_bass_kernel_spmd(nc, in_maps, list(range(NCORES))).results

    out = np.zeros((B, S, D), dtype=np.float32)
    k = np.empty((B, H, S, DK), dtype=np.float32)
    v = np.empty((B, H, S, DK), dtype=np.float32)
    for c in range(NCORES):
        b = c // 4
        g = c % 4
        r = res[c]
        out[b] += np.asarray(r["ft_out"]).T
        kt = np.asarray(r["kt_out"])                                      # [256, S]
        k[b, 4 * g:4 * g + 4] = kt.reshape(4, DK, S).transpose(0, 2, 1)
        vv = np.asarray(r["v_out"]).reshape(128, RB, HPC, DK)
        v[b, 4 * g:4 * g + 4] = vv.transpose(2, 1, 0, 3).reshape(HPC, S, DK)
    return out, (k, v)
